# revision 4
# baseline (speedup 1.0000x reference)
"""Trainium2 Bass kernel for nn_LCritic (LSTM-cell critic forward).

Sharding: pure data parallel — batch B=131072 split across 8 NeuronCores
(16384 rows each); tiny weights replicated.

Device layout: everything feature-major ("transposed"), so all GEMMs have
the contraction dim on SBUF partitions with zero on-chip transposes.  The
host marshals inputs into feature-major per-core arrays (numpy), which is
part of input sharding/marshaling, not device time.

Per-core pipeline, per unit of 1024 batch columns (chunks lo/hi of 512):
  DMA:  sT (225 x B feature-major state|action), hT, cT (stacked pairs)
  PE:   x.T = W1.T @ sT        (PSUM, K=224 in 2 chunks)
  DVE:  xh[0:64] = relu(x.T + b1)        (fused bias+relu, PSUM->SBUF)
  PE:   per-gate stacked psum: P_g = [gate(lo); gate(hi)]  (K=128)
  ACT:  sigmoid/tanh with per-partition bias -> Si,Sf,Tg,So
  DVE:  c = Si*Tg + Sf*cT ; h = So*tanh(c)   (stacked [128,512] tiles)
  PE:   head out = blockdiag(W_out,W_out).T @ h   ([2,512] psum)
  ACT:  tanh(+b_out) -> out
  DMA:  store c.T, h.T, out
"""

import sys

if "/opt/trn_rl_repo" not in sys.path:
    sys.path.insert(0, "/opt/trn_rl_repo")

import numpy as np

B = 131072
H = 64
IN = 224
N_CORES = 8
BL = B // N_CORES  # 16384
U = 1024           # batch columns per unit
NU = BL // U       # 16 units

_prog_cache = {}


def _split_waits(nc, mybir, bass_rust, max_waits=1):
    """This walrus build accepts only one sem-wait per instruction; Tile can
    emit several.  Move extras onto standalone wait instructions."""
    n = 0
    for bb in nc.main_func.blocks:
        new = []
        for ins in bb.instructions:
            si = ins.sync_info
            if si is not None and len(si.on_wait) > max_waits:
                waits = list(si.on_wait)
                keep, extra = waits[:max_waits], waits[max_waits:]
                for i in range(0, len(extra), max_waits):
                    n += 1
                    new.append(
                        mybir.InstEventSemaphore(
                            name=f"WSPLIT-{n}",
                            engine=ins.engine,
                            ins=[],
                            outs=[],
                            sync_info=bass_rust.SyncInfo(
                                on_wait=extra[i : i + max_waits], on_update=[]
                            ),
                        )
                    )
                ins.sync_info = bass_rust.SyncInfo(
                    on_wait=keep, on_update=list(si.on_update)
                )
            new.append(ins)
        bb.instructions[:] = new


def build_program():
    """Trace the per-core Bass program (identical on all 8 cores)."""
    if "nc" in _prog_cache:
        return _prog_cache["nc"]

    import bass_rust
    import concourse.bass as bass
    import concourse.mybir as mybir
    import concourse.tile as tile

    dt = mybir.dt.float32
    AF = mybir.ActivationFunctionType
    OP = mybir.AluOpType

    nc = bass.Bass()
    sT = nc.dram_tensor("sT", [IN, BL], dt, kind="ExternalInput")
    hT = nc.dram_tensor("hT", [H, BL], dt, kind="ExternalInput")
    cT = nc.dram_tensor("cT", [H, BL], dt, kind="ExternalInput")
    W1 = nc.dram_tensor("W1", [IN, H], dt, kind="ExternalInput")
    B1 = nc.dram_tensor("B1", [H, 1], dt, kind="ExternalInput")
    W2 = nc.dram_tensor("W2", [2 * H, 4 * H], dt, kind="ExternalInput")
    B2 = nc.dram_tensor("B2", [2 * H, 4], dt, kind="ExternalInput")
    W3 = nc.dram_tensor("W3", [2 * H, 2], dt, kind="ExternalInput")
    B3 = nc.dram_tensor("B3", [2, 1], dt, kind="ExternalInput")
    hO = nc.dram_tensor("hO", [H, BL], dt, kind="ExternalOutput")
    cO = nc.dram_tensor("cO", [H, BL], dt, kind="ExternalOutput")
    oO = nc.dram_tensor("oO", [BL], dt, kind="ExternalOutput")

    with tile.TileContext(nc) as tc:
        with (
            tc.tile_pool(name="const", bufs=1) as cpool,
            tc.tile_pool(name="io", bufs=3) as iop,
            tc.tile_pool(name="mid", bufs=2) as midp,
            tc.tile_pool(name="psA", bufs=2, space="PSUM") as psA,
            tc.tile_pool(name="psB", bufs=1, space="PSUM") as psB,
        ):
            w1a = cpool.tile([128, H], dt)
            nc.sync.dma_start(w1a[:], W1[0:128, :])
            w1b = cpool.tile([IN - 128, H], dt)
            nc.sync.dma_start(w1b[:], W1[128:IN, :])
            w2t = cpool.tile([128, 4 * H], dt)
            nc.sync.dma_start(w2t[:], W2[:])
            w3t = cpool.tile([128, 2], dt)
            nc.sync.dma_start(w3t[:], W3[:])
            b1t = cpool.tile([H, 1], dt)
            nc.sync.dma_start(b1t[:], B1[:])
            b2t = cpool.tile([128, 4], dt)
            nc.sync.dma_start(b2t[:], B2[:])
            b3t = cpool.tile([2, 1], dt)
            nc.sync.dma_start(b3t[:], B3[:])

            for u in range(NU):
                c0 = u * U
                csl = slice(c0, c0 + U)
                sTa = iop.tile([128, U], dt, tag="sTa")
                nc.sync.dma_start(sTa[:], sT[0:128, csl])
                sTb = iop.tile([IN - 128, U], dt, tag="sTb")
                nc.sync.dma_start(sTb[:], sT[128:IN, csl])
                xh = iop.tile([128, U], dt, tag="xh")
                nc.sync.dma_start(xh[64:128, :], hT[:, csl])
                ct = iop.tile([128, U // 2], dt, tag="ct")
                nc.sync.dma_start(ct[0:64, :], cT[:, c0 : c0 + 512])
                nc.sync.dma_start(ct[64:128, :], cT[:, c0 + 512 : c0 + U])

                # input projection + fused bias+relu
                for n in range(2):
                    ns = slice(n * 512, (n + 1) * 512)
                    px = psA.tile([H, 512], dt, tag="px")
                    nc.tensor.matmul(px[:], w1a[:], sTa[:, ns], start=True, stop=False)
                    nc.tensor.matmul(px[:], w1b[:], sTb[:, ns], start=False, stop=True)
                    nc.vector.tensor_scalar(
                        xh[0:64, ns], px[:], b1t[:], 0.0, OP.add, OP.max
                    )

                # fused gates: stacked per-gate psum [g(lo); g(hi)]
                pg = []
                for g in range(4):
                    p = psB.tile([128, 512], dt, tag=f"pg{g}")
                    ws = w2t[:, g * 64 : (g + 1) * 64]
                    nc.tensor.matmul(p[0:64, :], ws, xh[:, 0:512], start=True, stop=True)
                    nc.tensor.matmul(
                        p[64:128, :], ws, xh[:, 512:1024], start=True, stop=True
                    )
                    pg.append(p)

                si = midp.tile([128, 512], dt, tag="si")
                nc.scalar.activation(si[:], pg[0][:], AF.Sigmoid, bias=b2t[:, 0:1])
                sf = midp.tile([128, 512], dt, tag="sf")
                nc.scalar.activation(sf[:], pg[1][:], AF.Sigmoid, bias=b2t[:, 1:2])
                tg = midp.tile([128, 512], dt, tag="tg")
                nc.scalar.activation(tg[:], pg[2][:], AF.Tanh, bias=b2t[:, 2:3])
                so = midp.tile([128, 512], dt, tag="so")
                nc.scalar.activation(so[:], pg[3][:], AF.Sigmoid, bias=b2t[:, 3:4])

                a1 = midp.tile([128, 512], dt, tag="a1")
                nc.vector.tensor_mul(a1[:], si[:], tg[:])
                a2 = midp.tile([128, 512], dt, tag="a2")
                nc.vector.tensor_mul(a2[:], sf[:], ct[:])
                cst = midp.tile([128, 512], dt, tag="cst")
                nc.vector.tensor_add(cst[:], a1[:], a2[:])
                nc.sync.dma_start(cO[:, c0 : c0 + 512], cst[0:64, :])
                nc.sync.dma_start(cO[:, c0 + 512 : c0 + U], cst[64:128, :])
                tct = midp.tile([128, 512], dt, tag="tct")
                nc.scalar.activation(tct[:], cst[:], AF.Tanh)
                hst = midp.tile([128, 512], dt, tag="hst")
                nc.vector.tensor_mul(hst[:], so[:], tct[:])
                nc.sync.dma_start(hO[:, c0 : c0 + 512], hst[0:64, :])
                nc.sync.dma_start(hO[:, c0 + 512 : c0 + U], hst[64:128, :])

                # head: block-diag [W_out|0 ; 0|W_out] gives both stacked
                # halves in one matmul -> [2, 512] psum
                po = psB.tile([2, 512], dt, tag="po")
                nc.tensor.matmul(po[:], w3t[:], hst[:], start=True, stop=True)
                ot = midp.tile([2, 512], dt, tag="ot")
                nc.scalar.activation(ot[:], po[:], AF.Tanh, bias=b3t[:])
                nc.sync.dma_start(oO[csl].rearrange("(r n) -> r n", r=2), ot[:])

    _split_waits(nc, mybir, bass_rust)
    _prog_cache["nc"] = nc
    return nc


def prep_in_maps(state, action, hidden, cell, W_in, b_in, W_i, b_i, W_h, b_h,
                 W_out, b_out):
    """Shard + marshal full inputs into per-core feature-major arrays."""
    f32 = np.float32
    x = np.concatenate([state, action], axis=1)  # [B, 224]
    W1 = np.ascontiguousarray(W_in, dtype=f32)
    B1 = np.ascontiguousarray(b_in.reshape(H, 1), dtype=f32)
    W2 = np.ascontiguousarray(np.concatenate([W_i, W_h], axis=0), dtype=f32)
    b2 = (b_i + b_h).astype(f32)
    B2 = np.empty((2 * H, 4), f32)
    for g in range(4):
        B2[0:H, g] = b2[g * H : (g + 1) * H]
        B2[H : 2 * H, g] = b2[g * H : (g + 1) * H]
    W3 = np.zeros((2 * H, 2), f32)
    W3[0:H, 0] = W_out[:, 0]
    W3[H : 2 * H, 1] = W_out[:, 0]
    B3 = np.full((2, 1), b_out[0], f32)

    in_maps = []
    for c in range(N_CORES):
        sl = slice(c * BL, (c + 1) * BL)
        in_maps.append(
            {
                "sT": np.ascontiguousarray(x[sl].T, dtype=f32),
                "hT": np.ascontiguousarray(hidden[sl].T, dtype=f32),
                "cT": np.ascontiguousarray(cell[sl].T, dtype=f32),
                "W1": W1, "B1": B1, "W2": W2, "B2": B2, "W3": W3, "B3": B3,
            }
        )
    return in_maps


def assemble_outputs(results):
    """Gather per-core feature-major outputs into the reference's tuple."""
    f32 = np.float32
    out = np.concatenate([r["oO"] for r in results]).reshape(B, 1).astype(f32)
    h = np.ascontiguousarray(
        np.concatenate([r["hO"].T for r in results], axis=0), dtype=f32
    )
    c = np.ascontiguousarray(
        np.concatenate([r["cO"].T for r in results], axis=0), dtype=f32
    )
    h_r = h.reshape(2, 1, -1)
    c_r = c.reshape(2, 1, -1)
    return (out, h_r[0], h_r[1], c_r[0], c_r[1])


def kernel(state, action, hidden, cell, W_in, b_in, W_i, b_i, W_h, b_h,
           W_out, b_out):
    from concourse.bass_utils import run_bass_kernel_spmd

    nc = build_program()
    in_maps = prep_in_maps(state, action, hidden, cell, W_in, b_in, W_i, b_i,
                           W_h, b_h, W_out, b_out)
    res = run_bass_kernel_spmd(nc, in_maps, core_ids=list(range(N_CORES)))
    return assemble_outputs(res.results)
